# revision 7
# baseline (speedup 1.0000x reference)
"""Trainium2 Bass kernel for CombinedBandPassFilterSequential.

Zero-phase (filtfilt-style) FIR filter bank: 10 phase bands (K=769) +
10 amplitude bands (K=129) over a single (1,1,2097152) fp32 signal;
output is the 20 band signals concatenated on the last axis.

Strategy
--------
Time-sharded SPMD over 8 NeuronCores: each core processes a contiguous
T/8 slice of the signal for ALL 20 bands (perfect load balance).

The two-pass zero-phase filter equals a single cross-correlation with
g = autocorr(h) (2K-1 taps) everywhere except the first/last (K-1)/2
samples of the GLOBAL sequence; those few samples are computed exactly
on the host (numpy, float64) and spliced into the result. This fuses
the reference's two conv passes into one dense pass with no
intermediate staging and no edge masks on the device.

Each 1-D correlation is cast as a sequence of 128x128 @ 128x512
tensor-engine matmuls using banded-Toeplitz weight chunks
(PSUM accumulates fp32):

  out[128*i + r] = sum_q  W_q[:, r] . x_cols[:, i + q - Q0]

where x_cols[p, m] = x[128*m + p] is the signal in "transposed" column
layout (prepared on host) and W_q[p, r] = g[128*(q-Q0) + p - r + c].

Mixed precision, tuned so worst-case rel err stays ~2.5x under the
2e-2 gate: the large inner taps run in bf16 (1 col/cycle on the PE);
the small outer taps run as fp8e4 DoubleRow pairs (256-wide
contraction per instruction = 2 chunks per 512 cycles, 2x bf16
throughput; quantization noise is proportional to the small tail-tap
norm). pha: 9 bf16 chunks + 2 fp8 pairs (vs 13 bf16); amp: 1 bf16
chunk + 1 fp8 pair (vs 3 bf16). DoubleRow rhs slots read the signal at
block offsets (m, m+1) for consecutive-chunk pairs and (m, m+2) for
the amp (-1,+1) pair, via two shifted fp8 copies of x.
"""
import numpy as np
import ml_dtypes

import concourse.bass as bass
import concourse.tile as tile
from concourse import bacc, mybir
from concourse import bass_utils

# ---- problem geometry (hardcoded per contest rules) ----
T = 2097152
NCORES = 8
L = T // NCORES          # 262144 samples per core
LC = L // 128            # 2048 output columns per core
XH = 6                   # x halo columns each side (= pha (K-1)/128)
XC = LC + 2 * XH         # 2060 x columns
NB = 10                  # bands per filter group
QP, Q0P = 13, 6          # pha fused autocorr (1537 taps): chunk count, offset
QA, Q0A = 3, 1           # amp fused autocorr (257 taps): chunk count, offset
PB = 9                   # pha bf16 chunks per band (q' in [-4, 4])
CP = 384                 # pha edge-splice width ((K-1)/2)
CA = 64                  # amp edge-splice width
N = 512                  # matmul moving width (1 PSUM bank)
NG = LC // N             # 4 groups per band

F32 = mybir.dt.float32
BF16 = mybir.dt.bfloat16
FP8 = mybir.dt.float8e4
BF16_NP = ml_dtypes.bfloat16
FP8_NP = ml_dtypes.float8_e4m3
DR = mybir.MatmulPerfMode.DoubleRow


def _toeplitz_chunks(g, Q0, NQ):
    """W[q][p, r] = g[128*(q - Q0) + p - r + c], zero outside [0, len(g))."""
    g = np.asarray(g, np.float64)
    K = len(g)
    c = (K - 1) // 2
    W = np.zeros((NQ, 128, 128), np.float64)
    p = np.arange(128)[:, None]
    r = np.arange(128)[None, :]
    for q in range(NQ):
        k = 128 * (q - Q0) + p - r + c
        valid = (k >= 0) & (k < K)
        W[q][valid] = g[np.clip(k, 0, K - 1)][valid]
    return W


def _build_program():
    nc = bacc.Bacc("TRN2", target_bir_lowering=False, debug=False,
                   enable_asserts=True, num_devices=NCORES)

    x_ap = nc.dram_tensor("xT", [128, XC], BF16, kind="ExternalInput").ap()
    xd_ap = nc.dram_tensor("xd8", [128, 2, XC], FP8, kind="ExternalInput").ap()
    xe_ap = nc.dram_tensor("xe8", [128, 2, XC], FP8, kind="ExternalInput").ap()
    wp_ap = nc.dram_tensor("wp", [128, NB * PB * 128], BF16,
                           kind="ExternalInput").ap()
    wp8_ap = nc.dram_tensor("wp8", [128, NB * 4, 128], FP8,
                            kind="ExternalInput").ap()
    wa_ap = nc.dram_tensor("wa", [128, NB * 128], BF16,
                           kind="ExternalInput").ap()
    wa8_ap = nc.dram_tensor("wa8", [128, NB * 2, 128], FP8,
                            kind="ExternalInput").ap()
    out_ap = nc.dram_tensor("out", [2 * NB, 128, LC], BF16,
                            kind="ExternalOutput").ap()

    with tile.TileContext(nc) as tc:
        with tc.tile_pool(name="const", bufs=1) as cpool, \
             tc.tile_pool(name="psum", bufs=8, space="PSUM") as psum_pool, \
             tc.tile_pool(name="stage", bufs=4) as stage_pool:

            xt = cpool.tile([128, XC], BF16, name="xt", tag="xT")
            xd8 = cpool.tile([128, 2, XC], FP8, name="xdt", tag="xd8")
            xe8 = cpool.tile([128, 2, XC], FP8, name="xet", tag="xe8")
            wp = cpool.tile([128, NB * PB * 128], BF16, name="wpt", tag="wp")
            wp8 = cpool.tile([128, NB * 4, 128], FP8, name="wp8t", tag="wp8")
            wa = cpool.tile([128, NB * 128], BF16, name="wat", tag="wa")
            wa8 = cpool.tile([128, NB * 2, 128], FP8, name="wa8t", tag="wa8")

            # Input DMAs spread across the ACT and GPSIMD rings so the
            # per-queue ~0.6us issue cost and transfers overlap; the SP
            # ring is reserved for output stores. Ordered by first use:
            # the PE's first matmul (bf16 chunk 0 of band 0) waits only on
            # ~0.23 MB; each chain runs its bf16 chunks first so the fp8
            # tensors get ~2.4 us of extra landing slack.
            nc.scalar.dma_start(xt[:, 0:524], x_ap[:, 0:524])
            nc.scalar.dma_start(wp[:, 0:3 * 128], wp_ap[:, 0:3 * 128])
            nc.gpsimd.dma_start(wp8[:, 0:4, :], wp8_ap[:, 0:4, :])
            nc.scalar.dma_start(wp[:, 3 * 128:PB * 128], wp_ap[:, 3 * 128:PB * 128])
            nc.gpsimd.dma_start(xd8[:, :, 0:516], xd_ap[:, :, 0:516])
            nc.scalar.dma_start(xt[:, 524:1036], x_ap[:, 524:1036])
            nc.gpsimd.dma_start(xe8[:, :, 0:520], xe_ap[:, :, 0:520])
            nc.gpsimd.dma_start(xd8[:, :, 516:1040], xd_ap[:, :, 516:1040])
            nc.gpsimd.dma_start(wa8[:], wa8_ap[:])
            nc.gpsimd.dma_start(wa[:], wa_ap[:])
            nc.scalar.dma_start(xt[:, 1036:1548], x_ap[:, 1036:1548])
            nc.scalar.dma_start(xt[:, 1548:XC], x_ap[:, 1548:XC])
            nc.gpsimd.dma_start(xd8[:, :, 1040:XC], xd_ap[:, :, 1040:XC])
            nc.gpsimd.dma_start(xe8[:, :, 520:XC], xe_ap[:, :, 520:XC])
            nc.gpsimd.dma_start(wp8[:, 4:NB * 4, :], wp8_ap[:, 4:NB * 4, :])

            def wslice(eng, tile_, ap, b, nq):
                s = b * nq * 128
                e = (b + 1) * nq * 128
                eng.dma_start(tile_[:, s:e], ap[:, s:e])

            for b in range(1, NB):
                wslice(nc.scalar if b % 2 else nc.gpsimd, wp, wp_ap, b, PB)

            def pha_band(b):
                st = None
                for g in range(NG):
                    i0 = g * N
                    ps = psum_pool.tile([128, N], F32, tag="ps")
                    # bf16 chunks q' = -4 .. 4
                    for j in range(PB):
                        m0 = i0 + 2 + j
                        nc.tensor.matmul(
                            ps[:],
                            wp[:, (b * PB + j) * 128:(b * PB + j + 1) * 128],
                            xt[:, m0:m0 + N],
                            start=(j == 0), stop=False)
                    # fp8 pairs: chunks q' = (-6,-5), (+5,+6)
                    nc.tensor.matmul(
                        ps[:], wp8[:, b * 4:b * 4 + 2, :],
                        xd8[:, :, i0:i0 + N],
                        start=False, stop=False, perf_mode=DR)
                    nc.tensor.matmul(
                        ps[:], wp8[:, b * 4 + 2:b * 4 + 4, :],
                        xd8[:, :, i0 + 11:i0 + 11 + N],
                        start=False, stop=True, perf_mode=DR)
                    st = _drain(st, ps, g, b, False)

            def amp_band(b, last=False):
                st = None
                for g in range(NG):
                    i0 = g * N
                    ps = psum_pool.tile([128, N], F32, tag="ps")
                    # bf16 center chunk q' = 0
                    nc.tensor.matmul(
                        ps[:], wa[:, b * 128:(b + 1) * 128],
                        xt[:, i0 + 6:i0 + 6 + N],
                        start=True, stop=False)
                    # fp8 pair: chunks q' = -1, +1 (rhs blocks i-1, i+1)
                    nc.tensor.matmul(
                        ps[:], wa8[:, b * 2:b * 2 + 2, :],
                        xe8[:, :, i0 + 5:i0 + 5 + N],
                        start=False, stop=True, perf_mode=DR)
                    st = _drain(st, ps, g, NB + b, last)

            def _drain(st, ps, g, out_b, last):
                # 2 groups share one 1024-col bf16 stage -> 2KB DMA lines;
                # the final band stores per group so its last store's DMA
                # overlaps the preceding group's copy.
                i0 = g * N
                if last:
                    st = stage_pool.tile([128, N], BF16, tag="st")
                    if g % 2 == 0:
                        nc.vector.tensor_copy(st[:], ps[:])
                    else:
                        nc.scalar.copy(st[:], ps[:])
                    nc.sync.dma_start(out_ap[out_b, :, i0:i0 + N], st[:])
                elif g % 2 == 0:
                    st = stage_pool.tile([128, 2 * N], BF16, tag="st")
                    nc.vector.tensor_copy(st[:, :N], ps[:])
                else:
                    nc.scalar.copy(st[:, N:], ps[:])
                    nc.sync.dma_start(out_ap[out_b, :, i0 - N:i0 + N], st[:])
                return st

            # amp interleaved between pha bands: amp's 2-matmul groups
            # produce drains ~5x faster than pha's 11-matmul groups; the
            # mix keeps DVE/ACT drain demand under their throughput.
            for b in range(NB):
                pha_band(b)
                amp_band(b, last=(b == NB - 1))

    nc.compile()
    return nc


_CACHE = {}


def _get_program():
    if "nc" not in _CACHE:
        _CACHE["nc"] = _build_program()
    return _CACHE["nc"]


def _host_inputs(x, pha_filters, amp_filters):
    x = np.ascontiguousarray(np.asarray(x, np.float32).reshape(T))
    pha = np.asarray(pha_filters, np.float64)
    amp = np.asarray(amp_filters, np.float64)

    gp = [np.correlate(h, h, "full") for h in pha]   # 1537 taps
    ga = [np.correlate(h, h, "full") for h in amp]   # 257 taps
    Wp = np.stack([_toeplitz_chunks(g, Q0P, QP) for g in gp])  # (NB,13,128,128)
    Wa = np.stack([_toeplitz_chunks(g, Q0A, QA) for g in ga])  # (NB,3,128,128)

    def wlay(W, dt):  # (NB, NQ, 128p, 128r) -> (128p, NB*NQ*128r)
        return np.ascontiguousarray(
            W.transpose(2, 0, 1, 3).reshape(128, -1).astype(dt))

    wp = wlay(Wp[:, 2:11], BF16_NP)                      # q' in [-4, 4]
    wp8 = wlay(Wp[:, [0, 1, 11, 12]], FP8_NP)            # q' = -6,-5,+5,+6
    wa = wlay(Wa[:, 1:2], BF16_NP)                       # q' = 0
    wa8 = wlay(Wa[:, [0, 2]], FP8_NP)                    # q' = -1, +1

    xpad = np.zeros(T + (2 * XH + 2) * 128, np.float32)
    xpad[XH * 128: XH * 128 + T] = x

    in_maps = []
    for c in range(NCORES):
        n0 = c * L
        xcols = xpad[n0:n0 + (XC + 2) * 128].reshape(XC + 2, 128).T
        xt = np.ascontiguousarray(xcols[:, :XC].astype(BF16_NP))
        x8 = xcols.astype(FP8_NP)
        xd8 = np.ascontiguousarray(
            np.stack([x8[:, 0:XC], x8[:, 1:XC + 1]], axis=1))
        xe8 = np.ascontiguousarray(
            np.stack([x8[:, 0:XC], x8[:, 2:XC + 2]], axis=1))
        in_maps.append({"xT": xt, "xd8": xd8, "xe8": xe8,
                        "wp": wp, "wp8": wp8, "wa": wa, "wa8": wa8})
    return in_maps


def _edge_exact(x, h, W, win=3072):
    """Exact two-pass values for out[:W] and out[T-W:] (float64 host)."""
    K = len(h)
    c = (K - 1) // 2
    xs = x[:win]
    f1 = np.convolve(xs, h[::-1])
    y1 = f1[K - 1 - c:K - 1 - c + win]
    y1v = win - c
    f2 = np.convolve(y1[:y1v], h)
    head = f2[c:c + y1v - c][:W]
    xs = x[::-1][:win]
    f1 = np.convolve(xs, h)
    y1r = f1[c:c + win]
    f2 = np.convolve(y1r[:y1v], h[::-1])
    tail = f2[K - 1 - c:K - 1 - c + y1v - c][:W][::-1]
    return head, tail


def _gather(results, x, pha_filters, amp_filters):
    out = np.empty((2 * NB, T), np.float32)
    for c in range(NCORES):
        oc = np.asarray(results[c]["out"]).astype(np.float32)
        out[:, c * L:(c + 1) * L] = oc.transpose(0, 2, 1).reshape(2 * NB, L)
    # splice exact global-edge samples (fused autocorr differs from the
    # reference's cropped two-pass only within (K-1)/2 of each end)
    x64 = np.asarray(x, np.float64).reshape(T)
    for b in range(NB):
        head, tail = _edge_exact(x64, np.asarray(pha_filters[b], np.float64), CP)
        out[b, :CP] = head
        out[b, T - CP:] = tail
        head, tail = _edge_exact(x64, np.asarray(amp_filters[b], np.float64), CA)
        out[NB + b, :CA] = head
        out[NB + b, T - CA:] = tail
    return out.reshape(1, 1, 2 * NB * T)


def run(x, pha_filters, amp_filters, trace=False):
    nc = _get_program()
    in_maps = _host_inputs(x, pha_filters, amp_filters)
    res = bass_utils.run_bass_kernel_spmd(
        nc, in_maps, core_ids=list(range(NCORES)), trace=trace)
    return _gather(res.results, x, pha_filters, amp_filters), res


def kernel(x, pha_filters, amp_filters):
    out, _ = run(x, pha_filters, amp_filters)
    return out


# revision 10
# speedup vs baseline: 1.0012x; 1.0012x over previous
"""Trainium2 Bass kernel for CombinedBandPassFilterSequential.

Zero-phase (filtfilt-style) FIR filter bank: 10 phase bands (K=769) +
10 amplitude bands (K=129) over a single (1,1,2097152) fp32 signal;
output is the 20 band signals concatenated on the last axis.

Strategy
--------
Time-sharded SPMD over 8 NeuronCores: each core processes a contiguous
T/8 slice of the signal for ALL 20 bands (perfect load balance).

The two-pass zero-phase filter equals a single cross-correlation with
g = autocorr(h) (2K-1 taps) everywhere except the first/last (K-1)/2
samples of the GLOBAL sequence; those few samples are computed exactly
on the host (numpy, float64) and spliced into the result. This fuses
the reference's two conv passes into one dense pass with no
intermediate staging and no edge masks on the device.

Each 1-D correlation is cast as a sequence of 128x128 @ 128x512
tensor-engine matmuls using banded-Toeplitz weight chunks
(PSUM accumulates fp32):

  out[128*i + r] = sum_q  W_q[:, r] . x_cols[:, i + q - Q0]

where x_cols[p, m] = x[128*m + p] is the signal in "transposed" column
layout (prepared on host) and W_q[p, r] = g[128*(q-Q0) + p - r + c].

Mixed precision, tuned so worst-case rel err stays ~2.5x under the
2e-2 gate: the large inner taps run in bf16 (1 col/cycle on the PE);
the small outer taps run as fp8e4 DoubleRow pairs (256-wide
contraction per instruction = 2 chunks per 512 cycles, 2x bf16
throughput; quantization noise is proportional to the small tail-tap
norm). pha: 9 bf16 chunks + 2 fp8 pairs (vs 13 bf16); amp: 1 bf16
chunk + 1 fp8 pair (vs 3 bf16). DoubleRow rhs slots read the signal at
block offsets (m, m+1) for consecutive-chunk pairs and (m, m+2) for
the amp (-1,+1) pair, via two shifted fp8 copies of x.
"""
import numpy as np
import ml_dtypes

import concourse.bass as bass
import concourse.tile as tile
from concourse import bacc, mybir
from concourse import bass_utils

# ---- problem geometry (hardcoded per contest rules) ----
T = 2097152
NCORES = 8
L = T // NCORES          # 262144 samples per core
LC = L // 128            # 2048 output columns per core
XH = 6                   # x halo columns each side (= pha (K-1)/128)
XC = LC + 2 * XH         # 2060 x columns
NB = 10                  # bands per filter group
QP, Q0P = 13, 6          # pha fused autocorr (1537 taps): chunk count, offset
QA, Q0A = 3, 1           # amp fused autocorr (257 taps): chunk count, offset
PB = 9                   # pha bf16 chunks per band (q' in [-4, 4])
CP = 384                 # pha edge-splice width ((K-1)/2)
CA = 64                  # amp edge-splice width
N = 512                  # matmul moving width (1 PSUM bank)
NG = LC // N             # 4 groups per band

F32 = mybir.dt.float32
BF16 = mybir.dt.bfloat16
FP8 = mybir.dt.float8e4
BF16_NP = ml_dtypes.bfloat16
FP8_NP = ml_dtypes.float8_e4m3
DR = mybir.MatmulPerfMode.DoubleRow


def _toeplitz_chunks(g, Q0, NQ):
    """W[q][p, r] = g[128*(q - Q0) + p - r + c], zero outside [0, len(g))."""
    g = np.asarray(g, np.float64)
    K = len(g)
    c = (K - 1) // 2
    W = np.zeros((NQ, 128, 128), np.float64)
    p = np.arange(128)[:, None]
    r = np.arange(128)[None, :]
    for q in range(NQ):
        k = 128 * (q - Q0) + p - r + c
        valid = (k >= 0) & (k < K)
        W[q][valid] = g[np.clip(k, 0, K - 1)][valid]
    return W


def _build_program():
    nc = bacc.Bacc("TRN2", target_bir_lowering=False, debug=False,
                   enable_asserts=False, num_devices=NCORES)

    x_ap = nc.dram_tensor("xT", [128, XC], BF16, kind="ExternalInput").ap()
    xd_ap = nc.dram_tensor("xd8", [128, 2, XC], FP8, kind="ExternalInput").ap()
    xe_ap = nc.dram_tensor("xe8", [128, 2, XC], FP8, kind="ExternalInput").ap()
    wp_ap = nc.dram_tensor("wp", [128, NB * PB * 128], BF16,
                           kind="ExternalInput").ap()
    wp8_ap = nc.dram_tensor("wp8", [128, NB * 4, 128], FP8,
                            kind="ExternalInput").ap()
    wa_ap = nc.dram_tensor("wa", [128, NB * 128], BF16,
                           kind="ExternalInput").ap()
    wa8_ap = nc.dram_tensor("wa8", [128, NB * 2, 128], FP8,
                            kind="ExternalInput").ap()
    out_ap = nc.dram_tensor("out", [2 * NB, 128, LC], BF16,
                            kind="ExternalOutput").ap()

    with tile.TileContext(nc) as tc:
        with tc.tile_pool(name="const", bufs=1) as cpool, \
             tc.tile_pool(name="psum", bufs=8, space="PSUM") as psum_pool, \
             tc.tile_pool(name="stage", bufs=4) as stage_pool:

            xt = cpool.tile([128, XC], BF16, name="xt", tag="xT")
            xd8 = cpool.tile([128, 2, XC], FP8, name="xdt", tag="xd8")
            xe8 = cpool.tile([128, 2, XC], FP8, name="xet", tag="xe8")
            wp = cpool.tile([128, NB * PB * 128], BF16, name="wpt", tag="wp")
            wp8 = cpool.tile([128, NB * 4, 128], FP8, name="wp8t", tag="wp8")
            wa = cpool.tile([128, NB * 128], BF16, name="wat", tag="wa")
            wa8 = cpool.tile([128, NB * 2, 128], FP8, name="wa8t", tag="wa8")

            # Input DMAs spread across the ACT and GPSIMD rings so the
            # per-queue ~0.6us issue cost and transfers overlap; the SP
            # ring is reserved for output stores. Ordered by first use:
            # the PE's first matmul (bf16 chunk 0 of band 0) waits only on
            # ~0.23 MB; each chain runs its bf16 chunks first so the fp8
            # tensors get ~2.4 us of extra landing slack.
            nc.scalar.dma_start(xt[:, 0:524], x_ap[:, 0:524])
            nc.scalar.dma_start(wp[:, 0:3 * 128], wp_ap[:, 0:3 * 128])
            nc.gpsimd.dma_start(wp8[:, 0:4, :], wp8_ap[:, 0:4, :])
            nc.scalar.dma_start(wp[:, 3 * 128:PB * 128], wp_ap[:, 3 * 128:PB * 128])
            nc.gpsimd.dma_start(xd8[:, :, 0:516], xd_ap[:, :, 0:516])
            nc.scalar.dma_start(xt[:, 524:1036], x_ap[:, 524:1036])
            nc.gpsimd.dma_start(xe8[:, :, 0:520], xe_ap[:, :, 0:520])
            nc.gpsimd.dma_start(xd8[:, :, 516:1040], xd_ap[:, :, 516:1040])
            nc.gpsimd.dma_start(wa8[:], wa8_ap[:])
            nc.gpsimd.dma_start(wa[:], wa_ap[:])
            nc.scalar.dma_start(xt[:, 1036:1548], x_ap[:, 1036:1548])
            nc.scalar.dma_start(xt[:, 1548:XC], x_ap[:, 1548:XC])
            nc.gpsimd.dma_start(xd8[:, :, 1040:XC], xd_ap[:, :, 1040:XC])
            nc.gpsimd.dma_start(xe8[:, :, 520:XC], xe_ap[:, :, 520:XC])
            nc.gpsimd.dma_start(wp8[:, 4:NB * 4, :], wp8_ap[:, 4:NB * 4, :])

            def wslice(eng, tile_, ap, b, nq):
                s = b * nq * 128
                e = (b + 1) * nq * 128
                eng.dma_start(tile_[:, s:e], ap[:, s:e])

            for b in range(1, NB):
                wslice(nc.scalar if b % 2 else nc.gpsimd, wp, wp_ap, b, PB)

            def pha_band(b):
                st = None
                for g in range(NG):
                    i0 = g * N
                    ps = psum_pool.tile([128, N], F32, tag="ps")
                    # bf16 chunks q' = -4 .. 4
                    for j in range(PB):
                        m0 = i0 + 2 + j
                        nc.tensor.matmul(
                            ps[:],
                            wp[:, (b * PB + j) * 128:(b * PB + j + 1) * 128],
                            xt[:, m0:m0 + N],
                            start=(j == 0), stop=False)
                    # fp8 pairs: chunks q' = (-6,-5), (+5,+6)
                    nc.tensor.matmul(
                        ps[:], wp8[:, b * 4:b * 4 + 2, :],
                        xd8[:, :, i0:i0 + N],
                        start=False, stop=False, perf_mode=DR)
                    nc.tensor.matmul(
                        ps[:], wp8[:, b * 4 + 2:b * 4 + 4, :],
                        xd8[:, :, i0 + 11:i0 + 11 + N],
                        start=False, stop=True, perf_mode=DR)
                    st = _drain(st, ps, g, b, False)

            def amp_band(b, last=False):
                st = None
                for g in range(NG):
                    i0 = g * N
                    ps = psum_pool.tile([128, N], F32, tag="ps")
                    # bf16 center chunk q' = 0
                    nc.tensor.matmul(
                        ps[:], wa[:, b * 128:(b + 1) * 128],
                        xt[:, i0 + 6:i0 + 6 + N],
                        start=True, stop=False)
                    # fp8 pair: chunks q' = -1, +1 (rhs blocks i-1, i+1)
                    nc.tensor.matmul(
                        ps[:], wa8[:, b * 2:b * 2 + 2, :],
                        xe8[:, :, i0 + 5:i0 + 5 + N],
                        start=False, stop=True, perf_mode=DR)
                    st = _drain(st, ps, g, NB + b, last)

            def _drain(st, ps, g, out_b, last):
                # all 4 groups of a band share one 2048-col bf16 stage ->
                # one 512KB store per band with 4KB DMA lines; the final
                # band stores per group so its last store's DMA overlaps
                # the preceding group's copy.
                i0 = g * N
                if last:
                    st = stage_pool.tile([128, N], BF16, tag="stl")
                    if g % 2 == 0:
                        nc.vector.tensor_copy(st[:], ps[:])
                    else:
                        nc.scalar.copy(st[:], ps[:])
                    nc.sync.dma_start(out_ap[out_b, :, i0:i0 + N], st[:])
                    return st
                if g == 0:
                    st = stage_pool.tile([128, LC], BF16, tag="st")
                if g % 2 == 0:
                    nc.vector.tensor_copy(st[:, i0:i0 + N], ps[:])
                else:
                    nc.scalar.copy(st[:, i0:i0 + N], ps[:])
                if g == NG - 1:
                    nc.sync.dma_start(out_ap[out_b, :, :], st[:])
                return st

            # amp interleaved between pha bands: amp's 2-matmul groups
            # produce drains ~5x faster than pha's 11-matmul groups; the
            # mix keeps DVE/ACT drain demand under their throughput.
            for b in range(NB):
                pha_band(b)
                amp_band(b, last=(b == NB - 1))

    nc.compile()
    return nc


_CACHE = {}


def _get_program():
    if "nc" not in _CACHE:
        _CACHE["nc"] = _build_program()
    return _CACHE["nc"]


def _host_inputs(x, pha_filters, amp_filters):
    x = np.ascontiguousarray(np.asarray(x, np.float32).reshape(T))
    pha = np.asarray(pha_filters, np.float64)
    amp = np.asarray(amp_filters, np.float64)

    gp = [np.correlate(h, h, "full") for h in pha]   # 1537 taps
    ga = [np.correlate(h, h, "full") for h in amp]   # 257 taps
    Wp = np.stack([_toeplitz_chunks(g, Q0P, QP) for g in gp])  # (NB,13,128,128)
    Wa = np.stack([_toeplitz_chunks(g, Q0A, QA) for g in ga])  # (NB,3,128,128)

    def wlay(W, dt):  # (NB, NQ, 128p, 128r) -> (128p, NB*NQ*128r)
        return np.ascontiguousarray(
            W.transpose(2, 0, 1, 3).reshape(128, -1).astype(dt))

    wp = wlay(Wp[:, 2:11], BF16_NP)                      # q' in [-4, 4]
    wp8 = wlay(Wp[:, [0, 1, 11, 12]], FP8_NP)            # q' = -6,-5,+5,+6
    wa = wlay(Wa[:, 1:2], BF16_NP)                       # q' = 0
    wa8 = wlay(Wa[:, [0, 2]], FP8_NP)                    # q' = -1, +1

    xpad = np.zeros(T + (2 * XH + 2) * 128, np.float32)
    xpad[XH * 128: XH * 128 + T] = x

    in_maps = []
    for c in range(NCORES):
        n0 = c * L
        xcols = xpad[n0:n0 + (XC + 2) * 128].reshape(XC + 2, 128).T
        xt = np.ascontiguousarray(xcols[:, :XC].astype(BF16_NP))
        x8 = xcols.astype(FP8_NP)
        xd8 = np.ascontiguousarray(
            np.stack([x8[:, 0:XC], x8[:, 1:XC + 1]], axis=1))
        xe8 = np.ascontiguousarray(
            np.stack([x8[:, 0:XC], x8[:, 2:XC + 2]], axis=1))
        in_maps.append({"xT": xt, "xd8": xd8, "xe8": xe8,
                        "wp": wp, "wp8": wp8, "wa": wa, "wa8": wa8})
    return in_maps


def _edge_exact(x, h, W, win=3072):
    """Exact two-pass values for out[:W] and out[T-W:] (float64 host)."""
    K = len(h)
    c = (K - 1) // 2
    xs = x[:win]
    f1 = np.convolve(xs, h[::-1])
    y1 = f1[K - 1 - c:K - 1 - c + win]
    y1v = win - c
    f2 = np.convolve(y1[:y1v], h)
    head = f2[c:c + y1v - c][:W]
    xs = x[::-1][:win]
    f1 = np.convolve(xs, h)
    y1r = f1[c:c + win]
    f2 = np.convolve(y1r[:y1v], h[::-1])
    tail = f2[K - 1 - c:K - 1 - c + y1v - c][:W][::-1]
    return head, tail


def _gather(results, x, pha_filters, amp_filters):
    out = np.empty((2 * NB, T), np.float32)
    for c in range(NCORES):
        oc = np.asarray(results[c]["out"]).astype(np.float32)
        out[:, c * L:(c + 1) * L] = oc.transpose(0, 2, 1).reshape(2 * NB, L)
    # splice exact global-edge samples (fused autocorr differs from the
    # reference's cropped two-pass only within (K-1)/2 of each end)
    x64 = np.asarray(x, np.float64).reshape(T)
    for b in range(NB):
        head, tail = _edge_exact(x64, np.asarray(pha_filters[b], np.float64), CP)
        out[b, :CP] = head
        out[b, T - CP:] = tail
        head, tail = _edge_exact(x64, np.asarray(amp_filters[b], np.float64), CA)
        out[NB + b, :CA] = head
        out[NB + b, T - CA:] = tail
    return out.reshape(1, 1, 2 * NB * T)


def run(x, pha_filters, amp_filters, trace=False):
    nc = _get_program()
    in_maps = _host_inputs(x, pha_filters, amp_filters)
    res = bass_utils.run_bass_kernel_spmd(
        nc, in_maps, core_ids=list(range(NCORES)), trace=trace)
    return _gather(res.results, x, pha_filters, amp_filters), res


def kernel(x, pha_filters, amp_filters):
    out, _ = run(x, pha_filters, amp_filters)
    return out


# revision 11
# speedup vs baseline: 1.0112x; 1.0100x over previous
"""Trainium2 Bass kernel for CombinedBandPassFilterSequential.

Zero-phase (filtfilt-style) FIR filter bank: 10 phase bands (K=769) +
10 amplitude bands (K=129) over a single (1,1,2097152) fp32 signal;
output is the 20 band signals concatenated on the last axis.

Strategy
--------
Time-sharded SPMD over 8 NeuronCores: each core processes a contiguous
T/8 slice of the signal for ALL 20 bands (perfect load balance).

The two-pass zero-phase filter equals a single cross-correlation with
g = autocorr(h) (2K-1 taps) everywhere except the first/last (K-1)/2
samples of the GLOBAL sequence; those few samples are computed exactly
on the host (numpy, float64) and spliced into the result. This fuses
the reference's two conv passes into one dense pass with no
intermediate staging and no edge masks on the device.

Each 1-D correlation is cast as a sequence of 128x128 @ 128x512
tensor-engine matmuls using banded-Toeplitz weight chunks
(PSUM accumulates fp32):

  out[128*i + r] = sum_q  W_q[:, r] . x_cols[:, i + q - Q0]

where x_cols[p, m] = x[128*m + p] is the signal in "transposed" column
layout (prepared on host) and W_q[p, r] = g[128*(q-Q0) + p - r + c].

Mixed precision, tuned so worst-case rel err stays ~2.5x under the
2e-2 gate: the large inner taps run in bf16 (1 col/cycle on the PE);
the small outer taps run as fp8e4 DoubleRow pairs (256-wide
contraction per instruction = 2 chunks per 512 cycles, 2x bf16
throughput; quantization noise is proportional to the small tail-tap
norm). pha: 9 bf16 chunks + 2 fp8 pairs (vs 13 bf16); amp: 1 bf16
chunk + 1 fp8 pair (vs 3 bf16). DoubleRow rhs slots read the signal at
block offsets (m, m+1) for consecutive-chunk pairs and (m, m+2) for
the amp (-1,+1) pair, via two shifted fp8 copies of x.
"""
import numpy as np
import ml_dtypes

import concourse.bass as bass
import concourse.tile as tile
from concourse import bacc, mybir
from concourse import bass_utils

# ---- problem geometry (hardcoded per contest rules) ----
T = 2097152
NCORES = 8
L = T // NCORES          # 262144 samples per core
LC = L // 128            # 2048 output columns per core
XH = 6                   # x halo columns each side (= pha (K-1)/128)
XC = LC + 2 * XH         # 2060 x columns
NB = 10                  # bands per filter group
QP, Q0P = 13, 6          # pha fused autocorr (1537 taps): chunk count, offset
QA, Q0A = 3, 1           # amp fused autocorr (257 taps): chunk count, offset
PB = 9                   # pha bf16 chunks per band (q' in [-4, 4])
CP = 384                 # pha edge-splice width ((K-1)/2)
CA = 64                  # amp edge-splice width
N = 512                  # matmul moving width (1 PSUM bank)
NG = LC // N             # 4 groups per band

F32 = mybir.dt.float32
BF16 = mybir.dt.bfloat16
FP8 = mybir.dt.float8e4
BF16_NP = ml_dtypes.bfloat16
FP8_NP = ml_dtypes.float8_e4m3
DR = mybir.MatmulPerfMode.DoubleRow


def _toeplitz_chunks(g, Q0, NQ):
    """W[q][p, r] = g[128*(q - Q0) + p - r + c], zero outside [0, len(g))."""
    g = np.asarray(g, np.float64)
    K = len(g)
    c = (K - 1) // 2
    W = np.zeros((NQ, 128, 128), np.float64)
    p = np.arange(128)[:, None]
    r = np.arange(128)[None, :]
    for q in range(NQ):
        k = 128 * (q - Q0) + p - r + c
        valid = (k >= 0) & (k < K)
        W[q][valid] = g[np.clip(k, 0, K - 1)][valid]
    return W


def _build_program():
    nc = bacc.Bacc("TRN2", target_bir_lowering=False, debug=False,
                   enable_asserts=False, num_devices=NCORES)

    x_ap = nc.dram_tensor("xT", [128, XC], BF16, kind="ExternalInput").ap()
    xd_ap = nc.dram_tensor("xd8", [128, 2, XC], FP8, kind="ExternalInput").ap()
    xe_ap = nc.dram_tensor("xe8", [128, 2, XC], FP8, kind="ExternalInput").ap()
    wp_ap = nc.dram_tensor("wp", [128, NB * PB * 128], BF16,
                           kind="ExternalInput").ap()
    wp8_ap = nc.dram_tensor("wp8", [128, NB * 4, 128], FP8,
                            kind="ExternalInput").ap()
    wa_ap = nc.dram_tensor("wa", [128, NB * 128], BF16,
                           kind="ExternalInput").ap()
    wa8_ap = nc.dram_tensor("wa8", [128, NB * 2, 128], FP8,
                            kind="ExternalInput").ap()
    out_ap = nc.dram_tensor("out", [2 * NB, 128, LC], BF16,
                            kind="ExternalOutput").ap()

    with tile.TileContext(nc) as tc:
        with tc.tile_pool(name="const", bufs=1) as cpool, \
             tc.tile_pool(name="psum", bufs=8, space="PSUM") as psum_pool, \
             tc.tile_pool(name="stage", bufs=4) as stage_pool:

            xt = cpool.tile([128, XC], BF16, name="xt", tag="xT")
            xd8 = cpool.tile([128, 2, XC], FP8, name="xdt", tag="xd8")
            xe8 = cpool.tile([128, 2, XC], FP8, name="xet", tag="xe8")
            wp = cpool.tile([128, NB * PB * 128], BF16, name="wpt", tag="wp")
            wp8 = cpool.tile([128, NB * 4, 128], FP8, name="wp8t", tag="wp8")
            wa = cpool.tile([128, NB * 128], BF16, name="wat", tag="wa")
            wa8 = cpool.tile([128, NB * 2, 128], FP8, name="wa8t", tag="wa8")

            # Input DMAs spread across the ACT and GPSIMD rings so the
            # per-queue ~0.6us issue cost and transfers overlap; the SP
            # ring is reserved for output stores. Ordered by first use:
            # the PE's first matmul (bf16 chunk 0 of band 0) waits only on
            # ~0.23 MB; each chain runs its bf16 chunks first so the fp8
            # tensors get ~2.4 us of extra landing slack.
            def wslice(eng, tile_, ap, b, nq):
                s = b * nq * 128
                e = (b + 1) * nq * 128
                eng.dma_start(tile_[:, s:e], ap[:, s:e])

            nc.scalar.dma_start(xt[:, 0:524], x_ap[:, 0:524])
            wslice(nc.scalar, wp, wp_ap, 0, PB)
            nc.gpsimd.dma_start(wp8[:, 0:4, :], wp8_ap[:, 0:4, :])
            nc.gpsimd.dma_start(xd8[:, :, 0:516], xd_ap[:, :, 0:516])
            nc.scalar.dma_start(xt[:, 524:1036], x_ap[:, 524:1036])
            nc.gpsimd.dma_start(xe8[:, :, 0:520], xe_ap[:, :, 0:520])
            nc.gpsimd.dma_start(xd8[:, :, 516:1040], xd_ap[:, :, 516:1040])
            nc.gpsimd.dma_start(wa8[:], wa8_ap[:])
            nc.gpsimd.dma_start(wa[:], wa_ap[:])
            nc.scalar.dma_start(xt[:, 1036:1548], x_ap[:, 1036:1548])
            nc.scalar.dma_start(xt[:, 1548:XC], x_ap[:, 1548:XC])
            nc.gpsimd.dma_start(xd8[:, :, 1040:XC], xd_ap[:, :, 1040:XC])
            nc.gpsimd.dma_start(xe8[:, :, 520:XC], xe_ap[:, :, 520:XC])
            nc.gpsimd.dma_start(wp8[:, 4:NB * 4, :], wp8_ap[:, 4:NB * 4, :])
            for b in range(1, NB):
                wslice(nc.scalar if b % 2 else nc.gpsimd, wp, wp_ap, b, PB)

            def pha_band(b):
                st = None
                for g in range(NG):
                    i0 = g * N
                    ps = psum_pool.tile([128, N], F32, tag="ps")
                    # bf16 chunks q' = -4 .. 4
                    for j in range(PB):
                        m0 = i0 + 2 + j
                        nc.tensor.matmul(
                            ps[:],
                            wp[:, (b * PB + j) * 128:(b * PB + j + 1) * 128],
                            xt[:, m0:m0 + N],
                            start=(j == 0), stop=False)
                    # fp8 pairs: chunks q' = (-6,-5), (+5,+6)
                    nc.tensor.matmul(
                        ps[:], wp8[:, b * 4:b * 4 + 2, :],
                        xd8[:, :, i0:i0 + N],
                        start=False, stop=False, perf_mode=DR)
                    nc.tensor.matmul(
                        ps[:], wp8[:, b * 4 + 2:b * 4 + 4, :],
                        xd8[:, :, i0 + 11:i0 + 11 + N],
                        start=False, stop=True, perf_mode=DR)
                    st = _drain(st, ps, g, b, False)

            def amp_band(b, last=False):
                st = None
                for g in range(NG):
                    i0 = g * N
                    ps = psum_pool.tile([128, N], F32, tag="ps")
                    # bf16 center chunk q' = 0
                    nc.tensor.matmul(
                        ps[:], wa[:, b * 128:(b + 1) * 128],
                        xt[:, i0 + 6:i0 + 6 + N],
                        start=True, stop=False)
                    # fp8 pair: chunks q' = -1, +1 (rhs blocks i-1, i+1)
                    nc.tensor.matmul(
                        ps[:], wa8[:, b * 2:b * 2 + 2, :],
                        xe8[:, :, i0 + 5:i0 + 5 + N],
                        start=False, stop=True, perf_mode=DR)
                    st = _drain(st, ps, g, NB + b, last)

            def _drain(st, ps, g, out_b, last):
                # all 4 groups of a band share one 2048-col bf16 stage ->
                # one 512KB store per band with 4KB DMA lines; the final
                # band stores per group so its last store's DMA overlaps
                # the preceding group's copy.
                i0 = g * N
                if last:
                    st = stage_pool.tile([128, N], BF16, tag="stl")
                    if g % 2 == 0:
                        nc.vector.tensor_copy(st[:], ps[:])
                    else:
                        nc.scalar.copy(st[:], ps[:])
                    nc.sync.dma_start(out_ap[out_b, :, i0:i0 + N], st[:])
                    return st
                if g == 0:
                    st = stage_pool.tile([128, LC], BF16, tag="st")
                if g % 2 == 0:
                    nc.vector.tensor_copy(st[:, i0:i0 + N], ps[:])
                else:
                    nc.scalar.copy(st[:, i0:i0 + N], ps[:])
                if g == NG - 1:
                    nc.sync.dma_start(out_ap[out_b, :, :], st[:])
                return st

            # amp interleaved between pha bands: amp's 2-matmul groups
            # produce drains ~5x faster than pha's 11-matmul groups; the
            # mix keeps DVE/ACT drain demand under their throughput.
            for b in range(NB):
                pha_band(b)
                amp_band(b, last=(b == NB - 1))

    nc.compile()
    return nc


_CACHE = {}


def _get_program():
    if "nc" not in _CACHE:
        _CACHE["nc"] = _build_program()
    return _CACHE["nc"]


def _host_inputs(x, pha_filters, amp_filters):
    x = np.ascontiguousarray(np.asarray(x, np.float32).reshape(T))
    pha = np.asarray(pha_filters, np.float64)
    amp = np.asarray(amp_filters, np.float64)

    gp = [np.correlate(h, h, "full") for h in pha]   # 1537 taps
    ga = [np.correlate(h, h, "full") for h in amp]   # 257 taps
    Wp = np.stack([_toeplitz_chunks(g, Q0P, QP) for g in gp])  # (NB,13,128,128)
    Wa = np.stack([_toeplitz_chunks(g, Q0A, QA) for g in ga])  # (NB,3,128,128)

    def wlay(W, dt):  # (NB, NQ, 128p, 128r) -> (128p, NB*NQ*128r)
        return np.ascontiguousarray(
            W.transpose(2, 0, 1, 3).reshape(128, -1).astype(dt))

    wp = wlay(Wp[:, 2:11], BF16_NP)                      # q' in [-4, 4]
    wp8 = wlay(Wp[:, [0, 1, 11, 12]], FP8_NP)            # q' = -6,-5,+5,+6
    wa = wlay(Wa[:, 1:2], BF16_NP)                       # q' = 0
    wa8 = wlay(Wa[:, [0, 2]], FP8_NP)                    # q' = -1, +1

    xpad = np.zeros(T + (2 * XH + 2) * 128, np.float32)
    xpad[XH * 128: XH * 128 + T] = x

    in_maps = []
    for c in range(NCORES):
        n0 = c * L
        xcols = xpad[n0:n0 + (XC + 2) * 128].reshape(XC + 2, 128).T
        xt = np.ascontiguousarray(xcols[:, :XC].astype(BF16_NP))
        x8 = xcols.astype(FP8_NP)
        xd8 = np.ascontiguousarray(
            np.stack([x8[:, 0:XC], x8[:, 1:XC + 1]], axis=1))
        xe8 = np.ascontiguousarray(
            np.stack([x8[:, 0:XC], x8[:, 2:XC + 2]], axis=1))
        in_maps.append({"xT": xt, "xd8": xd8, "xe8": xe8,
                        "wp": wp, "wp8": wp8, "wa": wa, "wa8": wa8})
    return in_maps


def _edge_exact(x, h, W, win=3072):
    """Exact two-pass values for out[:W] and out[T-W:] (float64 host)."""
    K = len(h)
    c = (K - 1) // 2
    xs = x[:win]
    f1 = np.convolve(xs, h[::-1])
    y1 = f1[K - 1 - c:K - 1 - c + win]
    y1v = win - c
    f2 = np.convolve(y1[:y1v], h)
    head = f2[c:c + y1v - c][:W]
    xs = x[::-1][:win]
    f1 = np.convolve(xs, h)
    y1r = f1[c:c + win]
    f2 = np.convolve(y1r[:y1v], h[::-1])
    tail = f2[K - 1 - c:K - 1 - c + y1v - c][:W][::-1]
    return head, tail


def _gather(results, x, pha_filters, amp_filters):
    out = np.empty((2 * NB, T), np.float32)
    for c in range(NCORES):
        oc = np.asarray(results[c]["out"]).astype(np.float32)
        out[:, c * L:(c + 1) * L] = oc.transpose(0, 2, 1).reshape(2 * NB, L)
    # splice exact global-edge samples (fused autocorr differs from the
    # reference's cropped two-pass only within (K-1)/2 of each end)
    x64 = np.asarray(x, np.float64).reshape(T)
    for b in range(NB):
        head, tail = _edge_exact(x64, np.asarray(pha_filters[b], np.float64), CP)
        out[b, :CP] = head
        out[b, T - CP:] = tail
        head, tail = _edge_exact(x64, np.asarray(amp_filters[b], np.float64), CA)
        out[NB + b, :CA] = head
        out[NB + b, T - CA:] = tail
    return out.reshape(1, 1, 2 * NB * T)


def run(x, pha_filters, amp_filters, trace=False):
    nc = _get_program()
    in_maps = _host_inputs(x, pha_filters, amp_filters)
    res = bass_utils.run_bass_kernel_spmd(
        nc, in_maps, core_ids=list(range(NCORES)), trace=trace)
    return _gather(res.results, x, pha_filters, amp_filters), res


def kernel(x, pha_filters, amp_filters):
    out, _ = run(x, pha_filters, amp_filters)
    return out
